# revision 10
# baseline (speedup 1.0000x reference)
"""CTC loss kernel for Trainium2 (8 NeuronCores, data-parallel over batch).

Math: with raw logits G[b,t,s] = pred[b,t,ext[b,s]] (ext = blank-interleaved
targets) the CTC forward recursion commutes with the per-frame log-softmax
normalizer: running the recursion on raw logits and subtracting
sum_t logsumexp_c(pred[b,t,:]) at the end gives the same loss. So the chip
computes (1) sum_c exp(pred) per (b,t) via streaming ACT exp+accumulate
(the memory-bound bulk) and (2) a probability-space forward recursion on the
VectorEngine with periodic renormalization; the recorded renorm multipliers
are compensated exactly on the host, which finishes the tiny scalar math in
float64.
"""

import sys

sys.path.insert(0, "/opt/trn_rl_repo")

import numpy as np

import concourse.bacc as bacc
import concourse.tile as tile
from concourse import mybir
from concourse.bass_utils import run_bass_kernel_spmd

B, T, C, L = 128, 160, 6625, 25
S = 2 * L + 1  # 51 CTC states
N_CORES = 8
BS = B // N_CORES  # 16 samples per core
TBLK = 8  # t-values per 128-row streaming block (8*16 = 128 rows)
NBLK = T // TBLK  # 20
# finer parts for the first/last streaming block: earlier pipeline start,
# smaller exposed tail.
QCHUNKS = [(0, 1657), (1657, 3313), (3313, 4969), (4969, 6625)]
NQCH = len(QCHUNKS)
QCHMAX = max(c1 - c0 for c0, c1 in QCHUNKS)
# one accumulator column per ACT accumulate: 4 chunks for block 0 (cols
# 0-3), one per middle block j=1..18 (col 3+j), 4 for block 19 (cols 22-25)
NACC = 2 * NQCH + (NBLK - 2)
NEG = -1.0e4  # exp() underflows to exactly 0.0f
RENORM_EVERY = 8
REN_STEPS = [t for t in range(1, T) if t % RENORM_EVERY == 0]
NREN = len(REN_STEPS)  # 19

f32 = mybir.dt.float32
f16 = mybir.dt.float16
Exp = mybir.ActivationFunctionType.Exp

_CACHE = {}


def _build_program():
    if "nc" in _CACHE:
        return _CACHE["nc"]
    nc = bacc.Bacc("TRN2", target_bir_lowering=False, debug=False,
                   num_devices=N_CORES)
    pred_d = nc.dram_tensor("pred", [BS, T, C], f32, kind="ExternalInput").ap()
    g_d = nc.dram_tensor("g", [BS, T * S], f32, kind="ExternalInput").ap()
    skip_d = nc.dram_tensor("skip", [BS, S], f32, kind="ExternalInput").ap()
    acc_d = nc.dram_tensor("acc", [128, NACC], f32,
                           kind="ExternalOutput").ap()
    afin_d = nc.dram_tensor("afin", [BS, S], f32, kind="ExternalOutput").ap()
    rnorm_d = nc.dram_tensor("rnorm", [BS, NREN], f32,
                             kind="ExternalOutput").ap()

    with tile.TileContext(nc) as tc:
        with (
            tc.tile_pool(name="persist", bufs=1) as pp,
            tc.tile_pool(name="steps", bufs=2) as stepp,
            tc.tile_pool(name="stream", bufs=4) as spool,
        ):
            # ---- recursion inputs (small), emitted first so ACT/DVE start
            # early. g/pt processed in quarters so the recursion can begin as
            # soon as the first quarter's exp lands (deps are range-granular).
            gt = pp.tile([BS, T * S], f32, tag="gt")
            pt = pp.tile([BS, T * S], f32, tag="pt")
            skipt = pp.tile([BS, S], f32, tag="skipt")
            quar = (T // 4) * S
            nc.sync.dma_start(out=skipt[:], in_=skip_d[:])
            for qi in range(4):
                a, b = qi * quar, (qi + 1) * quar
                nc.sync.dma_start(out=gt[:, a:b], in_=g_d[:, a:b])
            for qi in range(4):
                a, b = qi * quar, (qi + 1) * quar
                nc.scalar.activation(pt[:, a:b], gt[:, a:b], Exp)

            # ---- probability-space forward recursion, DVE only.
            # A tiles have 2 guard columns (always 0) so the s-1 / s-2 shifted
            # reads come from plain offset APs.
            Aa = pp.tile([BS, S + 2], f32, tag="Aa")
            Ab = pp.tile([BS, S + 2], f32, tag="Ab")
            Mt = pp.tile([BS, NREN], f32, tag="Mt")
            nc.vector.memset(Aa[:], 0.0)
            nc.vector.memset(Ab[:, 0:2], 0.0)
            # alpha0: states 0,1 get p[t=0, s=0,1], rest 0
            nc.vector.tensor_copy(out=Aa[:, 2:4], in_=pt[:, 0:2])

            cur, nxt = Aa, Ab
            k = 0
            for t in range(1, T):
                base = t * S
                u = stepp.tile([BS, S], f32, tag="u")
                v = stepp.tile([BS, S], f32, tag="v")
                # u = A[s] + A[s-1]
                nc.vector.tensor_add(out=u[:], in0=cur[:, 2:S + 2],
                                     in1=cur[:, 1:S + 1])
                # v = A[s-2] * skip_ok[s]
                nc.vector.tensor_mul(out=v[:], in0=cur[:, 0:S], in1=skipt[:])
                nc.vector.tensor_add(out=u[:], in0=u[:], in1=v[:])
                # A_new[s] = p_t[s] * (sum of paths)
                nc.vector.tensor_mul(out=nxt[:, 2:S + 2], in0=u[:],
                                     in1=pt[:, base:base + S])
                if t % RENORM_EVERY == 0:
                    mx = stepp.tile([BS, 1], f32, tag="mx")
                    nc.vector.reduce_max(mx[:], nxt[:, 2:S + 2],
                                         axis=mybir.AxisListType.X)
                    # record the actual multiplier used; host compensates with
                    # -log(r), so reciprocal accuracy does not matter.
                    nc.vector.reciprocal(out=Mt[:, k:k + 1], in_=mx[:])
                    nc.vector.tensor_scalar_mul(out=nxt[:, 2:S + 2],
                                                in0=nxt[:, 2:S + 2],
                                                scalar1=Mt[:, k:k + 1])
                    k += 1
                cur, nxt = nxt, cur
            assert k == NREN
            nc.sync.dma_start(out=afin_d[:], in_=cur[:, 2:S + 2])
            nc.sync.dma_start(out=rnorm_d[:], in_=Mt[:])

            # ---- streaming sum(exp(pred)) over C, 128 (b,t) rows per block.
            # The paired NeuronCore shares the 16 SBUF AXI ports (435 GB/s per
            # SEngine), capping plain fp32 streaming at ~217 GB/s/core. The
            # SWDGE inline fp32->fp16 cast halves the SBUF-write traffic, so
            # the HBM read side (~358 GB/s/core share) binds instead. exp is
            # computed in-place (elementwise); accumulation stays fp32.
            # Every accumulate targets its own column of one persistent tile:
            # no per-block memset/DMA lands in the Vector queue (which would
            # stall the serial recursion on buffer reuse), and the whole
            # accumulator ships in a single DMA at the end.
            accA = pp.tile([128, NACC], f32, tag="accA")
            for j in range(NBLK):
                src = pred_d[:, j * TBLK:(j + 1) * TBLK, :]
                if j in (0, NBLK - 1):
                    # chunked load into separate tiles for the first/last
                    # block: lets ACT start before the full block lands
                    # (first) and shortens the exposed ACT tail after the
                    # final transfer (last). Deps are tile-granular.
                    cb = 0 if j == 0 else NQCH + (NBLK - 2)
                    for ci, (c0, c1) in enumerate(QCHUNKS):
                        w = c1 - c0
                        if j == 0 and ci == 0:
                            # very first part rides the sync HWDGE ring as
                            # fp32: it starts ~5us before the SWDGE Q7
                            # pipeline warms up, and nothing else is writing
                            # SBUF that early so the doubled write width is
                            # free. Start latency is end latency when
                            # DMA-bound.
                            cp = spool.tile([128, QCHMAX], f32, tag="part32")
                            nc.sync.dma_start(out=cp[:, :w],
                                              in_=src[:, :, c0:c1])
                        else:
                            cp = spool.tile([128, QCHMAX], f16,
                                            tag="chunkpart")
                            nc.gpsimd.dma_start(out=cp[:, :w],
                                                in_=src[:, :, c0:c1])
                        nc.scalar.activation(cp[:, :w], cp[:, :w], Exp,
                                             accum_out=accA[:, cb + ci:
                                                            cb + ci + 1])
                else:
                    ct = spool.tile([128, C], f16, tag="chunk")
                    nc.gpsimd.dma_start(out=ct[:], in_=src)
                    nc.scalar.activation(ct[:], ct[:], Exp,
                                         accum_out=accA[:, NQCH + j - 1:
                                                        NQCH + j])
            nc.sync.dma_start(out=acc_d[:], in_=accA[:])

    nc.compile()
    _CACHE["nc"] = nc
    return nc


def prepare_in_maps(pred, targets, lens):
    """Host prep: extended labels, gathered logits G, skip mask; shard by core."""
    ext = np.zeros((B, S), dtype=np.int64)
    ext[:, 1::2] = targets
    G = pred[np.arange(B)[:, None, None], np.arange(T)[None, :, None],
             ext[:, None, :]]  # [B, T, S]
    valid = np.arange(S)[None, :] < (2 * lens + 1)[:, None]  # [B, S]
    G = np.where(valid[:, None, :], G, NEG).astype(np.float32)
    skip = np.pad((ext[:, 2:] != ext[:, :-2]) & (ext[:, 2:] != 0),
                  ((0, 0), (2, 0))).astype(np.float32)
    in_maps = []
    for c in range(N_CORES):
        sl = slice(c * BS, (c + 1) * BS)
        in_maps.append({
            "pred": np.ascontiguousarray(pred[sl]),
            "g": np.ascontiguousarray(G[sl].reshape(BS, T * S)),
            "skip": np.ascontiguousarray(skip[sl]),
        })
    return in_maps


def finish_host(results, lens):
    """Combine per-core outputs into the scalar mean loss (float64)."""
    loss_b = np.zeros(B, dtype=np.float64)
    with np.errstate(divide="ignore", invalid="ignore"):
        for c in range(N_CORES):
            r = results[c]
            acc = r["acc"].astype(np.float64)  # [128, NACC]
            ssum = np.empty((NBLK, 128))  # per-block row sums; row = b*8+t_off
            ssum[0] = acc[:, :NQCH].sum(-1)
            ssum[1:NBLK - 1] = acc[:, NQCH:NQCH + NBLK - 2].T
            ssum[NBLK - 1] = acc[:, NQCH + NBLK - 2:].sum(-1)
            lse = np.log(ssum)  # [NBLK, 128]
            s_lse = lse.reshape(NBLK, BS, TBLK).sum((0, 2))  # [BS]
            afin = r["afin"].astype(np.float64)  # [BS, S]
            rn = r["rnorm"].astype(np.float64)  # [BS, NREN]
            log_carry = np.log(rn).sum(1)  # [BS]
            for b in range(BS):
                gb = c * BS + b
                sE = 2 * int(lens[gb])
                le = np.logaddexp(np.log(afin[b, sE]), np.log(afin[b, sE - 1]))
                loss_b[gb] = s_lse[b] + log_carry[b] - le
    loss_b = np.where(loss_b >= 1e29, 0.0, loss_b)
    loss_b = np.where(np.isfinite(loss_b), loss_b, 0.0)
    loss = np.mean(loss_b / np.maximum(lens.astype(np.float64), 1.0))
    return np.float32(loss)


def kernel(pred, targets, targets_lengths):
    pred = np.asarray(pred, dtype=np.float32)
    targets = np.asarray(targets).astype(np.int64)
    lens = np.asarray(targets_lengths).astype(np.int64)

    nc = _build_program()
    in_maps = prepare_in_maps(pred, targets, lens)
    res = run_bass_kernel_spmd(nc, in_maps, core_ids=list(range(N_CORES)))
    return finish_host(res.results, lens)



# revision 15
# speedup vs baseline: 1.2071x; 1.2071x over previous
"""CTC loss kernel for Trainium2 (8 NeuronCores, data-parallel over batch).

Math: with raw logits G[b,t,s] = pred[b,t,ext[b,s]] (ext = blank-interleaved
targets) the CTC forward recursion commutes with the per-frame log-softmax
normalizer: running the recursion on raw logits and subtracting
sum_t logsumexp_c(pred[b,t,:]) at the end gives the same loss. So the chip
computes (1) sum_c exp(pred) per (b,t) via streaming ACT exp+accumulate
(the memory-bound bulk) and (2) a probability-space forward recursion on the
VectorEngine with periodic renormalization; the recorded renorm multipliers
are compensated exactly on the host, which finishes the tiny scalar math in
float64.
"""

import sys

sys.path.insert(0, "/opt/trn_rl_repo")

import numpy as np

import concourse.bacc as bacc
import concourse.tile as tile
from concourse import mybir
from concourse.bass_utils import run_bass_kernel_spmd

B, T, C, L = 128, 160, 6625, 25
S = 2 * L + 1  # 51 CTC states
N_CORES = 8
BS = B // N_CORES  # 16 samples per core
TBLK = 8  # t-values per 128-row streaming block (8*16 = 128 rows)
NBLK = T // TBLK  # 20
# finer parts for the first/last streaming block: earlier pipeline start,
# smaller exposed tail.
QCHUNKS = [(0, 1657), (1657, 3313), (3313, 4969), (4969, 6625)]
NQCH = len(QCHUNKS)
QCHMAX = max(c1 - c0 for c0, c1 in QCHUNKS)
# one accumulator column per ACT accumulate: 4 chunks for block 0 (cols
# 0-3), one per middle block j=1..18 (col 3+j), 4 for block 19 (cols 22-25)
NACC = 2 * NQCH + (NBLK - 2)
NEG = -1.0e4  # exp() underflows to exactly 0.0f
RENORM_EVERY = 8
REN_STEPS = [t for t in range(1, T) if t % RENORM_EVERY == 0]
NREN = len(REN_STEPS)  # 19

f32 = mybir.dt.float32
f16 = mybir.dt.float16
Exp = mybir.ActivationFunctionType.Exp

_CACHE = {}


def _build_program():
    if "nc" in _CACHE:
        return _CACHE["nc"]
    nc = bacc.Bacc("TRN2", target_bir_lowering=False, debug=False,
                   num_devices=N_CORES)
    pred_d = nc.dram_tensor("pred", [BS, T, C], f16, kind="ExternalInput").ap()
    g_d = nc.dram_tensor("g", [BS, T * S], f32, kind="ExternalInput").ap()
    skip_d = nc.dram_tensor("skip", [BS, S], f32, kind="ExternalInput").ap()
    acc_d = nc.dram_tensor("acc", [128, NACC], f32,
                           kind="ExternalOutput").ap()
    afin_d = nc.dram_tensor("afin", [BS, S], f32, kind="ExternalOutput").ap()
    rnorm_d = nc.dram_tensor("rnorm", [BS, NREN], f32,
                             kind="ExternalOutput").ap()

    with tile.TileContext(nc) as tc:
        with (
            tc.tile_pool(name="persist", bufs=1) as pp,
            tc.tile_pool(name="steps", bufs=2) as stepp,
            tc.tile_pool(name="stream", bufs=4) as spool,
        ):
            # ---- recursion inputs (small), emitted first so ACT/DVE start
            # early. g/pt processed in quarters so the recursion can begin as
            # soon as the first quarter's exp lands (deps are range-granular).
            gt = pp.tile([BS, T * S], f32, tag="gt")
            pt = pp.tile([BS, T * S], f32, tag="pt")
            skipt = pp.tile([BS, S], f32, tag="skipt")
            quar = (T // 4) * S
            nc.sync.dma_start(out=skipt[:], in_=skip_d[:])
            for qi in range(4):
                a, b = qi * quar, (qi + 1) * quar
                nc.sync.dma_start(out=gt[:, a:b], in_=g_d[:, a:b])
            for qi in range(4):
                a, b = qi * quar, (qi + 1) * quar
                nc.scalar.activation(pt[:, a:b], gt[:, a:b], Exp)

            # ---- probability-space forward recursion, DVE only.
            # A tiles have 2 guard columns (always 0) so the s-1 / s-2 shifted
            # reads come from plain offset APs.
            Aa = pp.tile([BS, S + 2], f32, tag="Aa")
            Ab = pp.tile([BS, S + 2], f32, tag="Ab")
            Mt = pp.tile([BS, NREN], f32, tag="Mt")
            nc.vector.memset(Aa[:], 0.0)
            nc.vector.memset(Ab[:, 0:2], 0.0)
            # alpha0: states 0,1 get p[t=0, s=0,1], rest 0
            nc.vector.tensor_copy(out=Aa[:, 2:4], in_=pt[:, 0:2])

            cur, nxt = Aa, Ab
            k = 0
            for t in range(1, T):
                base = t * S
                u = stepp.tile([BS, S], f32, tag="u")
                v = stepp.tile([BS, S], f32, tag="v")
                # u = A[s] + A[s-1]
                nc.vector.tensor_add(out=u[:], in0=cur[:, 2:S + 2],
                                     in1=cur[:, 1:S + 1])
                # v = A[s-2] * skip_ok[s]
                nc.vector.tensor_mul(out=v[:], in0=cur[:, 0:S], in1=skipt[:])
                nc.vector.tensor_add(out=u[:], in0=u[:], in1=v[:])
                # A_new[s] = p_t[s] * (sum of paths)
                nc.vector.tensor_mul(out=nxt[:, 2:S + 2], in0=u[:],
                                     in1=pt[:, base:base + S])
                if t % RENORM_EVERY == 0:
                    mx = stepp.tile([BS, 1], f32, tag="mx")
                    nc.vector.reduce_max(mx[:], nxt[:, 2:S + 2],
                                         axis=mybir.AxisListType.X)
                    # record the actual multiplier used; host compensates with
                    # -log(r), so reciprocal accuracy does not matter.
                    nc.vector.reciprocal(out=Mt[:, k:k + 1], in_=mx[:])
                    nc.vector.tensor_scalar_mul(out=nxt[:, 2:S + 2],
                                                in0=nxt[:, 2:S + 2],
                                                scalar1=Mt[:, k:k + 1])
                    k += 1
                cur, nxt = nxt, cur
            assert k == NREN

            # ---- streaming sum(exp(pred)) over C, 128 (b,t) rows per block.
            # pred is pre-cast to fp16 on the host (numerically identical to
            # the former in-DMA cast), so the stream needs no SWDGE: all loads
            # ride the sync HWDGE ring. SWDGE would stall here — its Q7
            # descriptor writes arbitrate for the DVE/GpSimd shared SBUF port
            # pair, which the now-continuously-busy recursion DVE holds ~95%
            # of the time. HWDGE descriptors are hardware-generated and
            # immune. fp16 halves HBM reads too; the binding limit becomes
            # the pair-shared SBUF AXI fabric (~435/2 GB/s per core).
            # Every accumulate targets its own column of one persistent tile:
            # no per-block memset/DMA lands in the Vector queue (which would
            # stall the serial recursion on buffer reuse), and the whole
            # accumulator ships in a single DMA at the end.
            accA = pp.tile([128, NACC], f32, tag="accA")
            for j in range(NBLK):
                src = pred_d[:, j * TBLK:(j + 1) * TBLK, :]
                if j in (0, NBLK - 1):
                    # chunked load into separate tiles for the first/last
                    # block: lets ACT start before the full block lands
                    # (first) and shortens the exposed ACT tail after the
                    # final transfer (last). Deps are tile-granular.
                    cb = 0 if j == 0 else NQCH + (NBLK - 2)
                    for ci, (c0, c1) in enumerate(QCHUNKS):
                        w = c1 - c0
                        cp = spool.tile([128, QCHMAX], f16, tag="chunkpart")
                        nc.sync.dma_start(out=cp[:, :w],
                                          in_=src[:, :, c0:c1])
                        nc.scalar.activation(cp[:, :w], cp[:, :w], Exp,
                                             accum_out=accA[:, cb + ci:
                                                            cb + ci + 1])
                else:
                    ct = spool.tile([128, C], f16, tag="chunk")
                    nc.sync.dma_start(out=ct[:], in_=src)
                    nc.scalar.activation(ct[:], ct[:], Exp,
                                         accum_out=accA[:, NQCH + j - 1:
                                                        NQCH + j])
            nc.sync.dma_start(out=acc_d[:], in_=accA[:])
            # recursion results ship after the stream DMAs are enqueued so
            # they cannot head-of-line-block the sync ring
            nc.sync.dma_start(out=afin_d[:], in_=cur[:, 2:S + 2])
            nc.sync.dma_start(out=rnorm_d[:], in_=Mt[:])

    nc.compile()
    _CACHE["nc"] = nc
    return nc


def prepare_in_maps(pred, targets, lens):
    """Host prep: extended labels, gathered logits G, skip mask; shard by core."""
    ext = np.zeros((B, S), dtype=np.int64)
    ext[:, 1::2] = targets
    G = pred[np.arange(B)[:, None, None], np.arange(T)[None, :, None],
             ext[:, None, :]]  # [B, T, S]
    valid = np.arange(S)[None, :] < (2 * lens + 1)[:, None]  # [B, S]
    G = np.where(valid[:, None, :], G, NEG).astype(np.float32)
    skip = np.pad((ext[:, 2:] != ext[:, :-2]) & (ext[:, 2:] != 0),
                  ((0, 0), (2, 0))).astype(np.float32)
    in_maps = []
    for c in range(N_CORES):
        sl = slice(c * BS, (c + 1) * BS)
        in_maps.append({
            "pred": np.ascontiguousarray(pred[sl]).astype(np.float16),
            "g": np.ascontiguousarray(G[sl].reshape(BS, T * S)),
            "skip": np.ascontiguousarray(skip[sl]),
        })
    return in_maps


def finish_host(results, lens):
    """Combine per-core outputs into the scalar mean loss (float64)."""
    loss_b = np.zeros(B, dtype=np.float64)
    with np.errstate(divide="ignore", invalid="ignore"):
        for c in range(N_CORES):
            r = results[c]
            acc = r["acc"].astype(np.float64)  # [128, NACC]
            ssum = np.empty((NBLK, 128))  # per-block row sums; row = b*8+t_off
            ssum[0] = acc[:, :NQCH].sum(-1)
            ssum[1:NBLK - 1] = acc[:, NQCH:NQCH + NBLK - 2].T
            ssum[NBLK - 1] = acc[:, NQCH + NBLK - 2:].sum(-1)
            lse = np.log(ssum)  # [NBLK, 128]
            s_lse = lse.reshape(NBLK, BS, TBLK).sum((0, 2))  # [BS]
            afin = r["afin"].astype(np.float64)  # [BS, S]
            rn = r["rnorm"].astype(np.float64)  # [BS, NREN]
            log_carry = np.log(rn).sum(1)  # [BS]
            for b in range(BS):
                gb = c * BS + b
                sE = 2 * int(lens[gb])
                le = np.logaddexp(np.log(afin[b, sE]), np.log(afin[b, sE - 1]))
                loss_b[gb] = s_lse[b] + log_carry[b] - le
    loss_b = np.where(loss_b >= 1e29, 0.0, loss_b)
    loss_b = np.where(np.isfinite(loss_b), loss_b, 0.0)
    loss = np.mean(loss_b / np.maximum(lens.astype(np.float64), 1.0))
    return np.float32(loss)


def kernel(pred, targets, targets_lengths):
    pred = np.asarray(pred, dtype=np.float32)
    targets = np.asarray(targets).astype(np.int64)
    lens = np.asarray(targets_lengths).astype(np.int64)

    nc = _build_program()
    in_maps = prepare_in_maps(pred, targets, lens)
    res = run_bass_kernel_spmd(nc, in_maps, core_ids=list(range(N_CORES)))
    return finish_host(res.results, lens)



# revision 20
# speedup vs baseline: 1.2859x; 1.0652x over previous
"""CTC loss kernel for Trainium2 (8 NeuronCores, data-parallel over batch).

Math: with raw logits G[b,t,s] = pred[b,t,ext[b,s]] (ext = blank-interleaved
targets) the CTC forward recursion commutes with the per-frame log-softmax
normalizer: running the recursion on raw logits and subtracting
sum_t logsumexp_c(pred[b,t,:]) at the end gives the same loss.

Device work, per core (16 samples):
1. sum_c exp(pred) per (b,t): fp8(e4m3) pred streamed via HWDGE, ACT
   exp+accumulate (ACT-throughput bound; fp8 host-cast keeps the DMA side
   far under the SBUF fabric share).
2. The CTC recursion in probability space, forward and backward chains in
   lockstep columns of one [102, 32] state: per super-step one PE matmul
   z = W^T X (W = [K1; K2] encodes the +0/+1/+2 state shifts, identical for
   both chains because the backward state is stored index-reversed) and two
   DVE multiplies by host-prepacked exp'd logits. Per-frame max-logit folding
   (host) replaces renormalization entirely; host compensates exactly.
Host finishes the tiny join + scalar math in float64.
"""

import sys

sys.path.insert(0, "/opt/trn_rl_repo")

import ml_dtypes
import numpy as np

import concourse.bacc as bacc
import concourse.tile as tile
from concourse import mybir
from concourse.bass_utils import run_bass_kernel_spmd

B, T, C, L = 128, 160, 6625, 25
S = 2 * L + 1  # 51 CTC states
SB2 = 2 * S  # 102: [y; yq] stacked state rows
N_CORES = 8
BS = B // N_CORES  # 16 samples per core
NCOL = 2 * BS  # 32: fwd cols 0-15, bwd cols 16-31
HS = T // 2  # 80 frames per chain
NSUP = HS - 1  # 79 super-steps
TBLK = 8  # t-values per 128-row streaming block (8*16 = 128 rows)
NBLK = T // TBLK  # 20
# finer parts for the first/last streaming block: earlier pipeline start,
# smaller exposed tail.
QCHUNKS = [(0, 1657), (1657, 3313), (3313, 4969), (4969, 6625)]
NQCH = len(QCHUNKS)
QCHMAX = max(c1 - c0 for c0, c1 in QCHUNKS)
# one accumulator column per ACT accumulate: 4 chunks for block 0 (cols
# 0-3), one per middle block j=1..18 (col 3+j), 4 for block 19 (cols 22-25)
NACC = 2 * NQCH + (NBLK - 2)
NEG = -1.0e4  # exp() underflows to exactly 0.0f
BOOST = 0.5  # per-frame fold = fmax - BOOST: keeps fp32 range centered

f32 = mybir.dt.float32
f16 = mybir.dt.float16
f8 = mybir.dt.float8e4
Exp = mybir.ActivationFunctionType.Exp
np_f8 = ml_dtypes.float8_e4m3

_CACHE = {}
_HOST = {}


def _build_program():
    if "nc" in _CACHE:
        return _CACHE["nc"]
    nc = bacc.Bacc("TRN2", target_bir_lowering=False, debug=False,
                   num_devices=N_CORES)
    pred_d = nc.dram_tensor("pred", [BS, T, C], f8, kind="ExternalInput").ap()
    w_d = nc.dram_tensor("w", [SB2, SB2], f32, kind="ExternalInput").ap()
    x0_d = nc.dram_tensor("x0", [SB2, NCOL], f16, kind="ExternalInput").ap()
    pq_d = nc.dram_tensor("pq", [SB2, NSUP * NCOL], f16,
                          kind="ExternalInput").ap()
    acc_d = nc.dram_tensor("acc", [128, NACC], f32,
                           kind="ExternalOutput").ap()
    xf_d = nc.dram_tensor("xfin", [SB2, NCOL], f32,
                          kind="ExternalOutput").ap()

    # pq column chunks (whole super-steps per chunk) for early recursion start
    PQCH = []
    st = 0
    for n in (20, 20, 20, 19):
        PQCH.append((st * NCOL, (st + n) * NCOL))
        st += n

    with tile.TileContext(nc) as tc:
        with (
            tc.tile_pool(name="persist", bufs=1) as pp,
            tc.tile_pool(name="stream", bufs=6) as spool,
            tc.tile_pool(name="zp", bufs=4, space="PSUM") as psp,
        ):
            # ---- recursion inputs, emitted first so PE/DVE start early
            wt = pp.tile([SB2, SB2], f32, tag="wt")
            x0l = pp.tile([SB2, NCOL], f16, tag="x0l")
            pqt = pp.tile([SB2, NSUP * NCOL], f16, tag="pqt")
            pqe = pp.tile([SB2, NSUP * NCOL], f32, tag="pqe")
            Xa = pp.tile([SB2, NCOL], f32, tag="Xa")
            Xb = pp.tile([SB2, NCOL], f32, tag="Xb")
            nc.sync.dma_start(out=wt[:], in_=w_d[:])
            nc.sync.dma_start(out=x0l[:], in_=x0_d[:])
            for a, b in PQCH:
                nc.sync.dma_start(out=pqt[:, a:b], in_=pq_d[:, a:b])
            nc.scalar.activation(Xa[:], x0l[:], Exp)
            for a, b in PQCH:
                nc.scalar.activation(pqe[:, a:b], pqt[:, a:b], Exp)

            # ---- lockstep fwd/bwd recursion: 79 x (1 matmul + 1 DVE mul).
            # The stationary matrix is [W | W] so the matmul lands z
            # duplicated on partitions 0-50 and 51-101 — DVE lanes cannot
            # cross partitions, so the y- and yq-halves each need z in their
            # own partitions; the duplicate makes the whole state update a
            # single partition-aligned multiply.
            cur, nxt = Xa, Xb
            for i in range(NSUP):
                z = psp.tile([SB2, NCOL], f32, tag="z")
                nc.tensor.matmul(z[:], wt[:], cur[:])
                c0 = i * NCOL
                nc.vector.tensor_mul(out=nxt[:], in0=z[:],
                                     in1=pqe[:, c0:c0 + NCOL])
                cur, nxt = nxt, cur

            # ---- streaming sum(exp(pred)) over C, 128 (b,t) rows per block.
            # pred is pre-cast to fp8 e4m3 on the host; all loads ride the
            # sync HWDGE ring (SWDGE would stall: its Q7 descriptor writes
            # arbitrate for the DVE/GpSimd shared SBUF port pair that the
            # recursion DVE holds most of the time; HWDGE is immune). The
            # stream is ACT-throughput bound (1 elem/cycle/lane), so the DMA
            # side has ample slack. exp output goes to one reused fp16
            # scratch (ACT engine port, free); accumulation stays fp32.
            # Every accumulate targets its own column of one persistent tile
            # and the whole accumulator ships in a single DMA at the end.
            accA = pp.tile([128, NACC], f32, tag="accA")
            scr = pp.tile([128, C], f16, tag="scr")
            for j in range(NBLK):
                src = pred_d[:, j * TBLK:(j + 1) * TBLK, :]
                if j in (0, NBLK - 1):
                    cb = 0 if j == 0 else NQCH + (NBLK - 2)
                    for ci, (c0, c1) in enumerate(QCHUNKS):
                        w = c1 - c0
                        cp = spool.tile([128, QCHMAX], f8, tag="chunkpart")
                        nc.sync.dma_start(out=cp[:, :w],
                                          in_=src[:, :, c0:c1])
                        nc.scalar.activation(scr[:, :w], cp[:, :w], Exp,
                                             accum_out=accA[:, cb + ci:
                                                            cb + ci + 1])
                else:
                    ct = spool.tile([128, C], f8, tag="chunk")
                    nc.sync.dma_start(out=ct[:], in_=src)
                    nc.scalar.activation(scr[:], ct[:], Exp,
                                         accum_out=accA[:, NQCH + j - 1:
                                                        NQCH + j])
            nc.sync.dma_start(out=acc_d[:], in_=accA[:])
            # recursion result ships after the stream DMAs are enqueued so it
            # cannot head-of-line-block the sync ring
            nc.sync.dma_start(out=xf_d[:], in_=cur[:])

    nc.compile()
    _CACHE["nc"] = nc
    return nc


def prepare_in_maps(pred, targets, lens):
    """Host prep: gathered+folded logit packs, per-core sharding."""
    ext = np.zeros((B, S), dtype=np.int64)
    ext[:, 1::2] = targets
    G = pred[np.arange(B)[:, None, None], np.arange(T)[None, :, None],
             ext[:, None, :]]  # [B, T, S]
    valid = np.arange(S)[None, :] < (2 * lens + 1)[:, None]  # [B, S]
    G = np.where(valid[:, None, :], G, NEG).astype(np.float32)
    skip = np.pad((ext[:, 2:] != ext[:, :-2]) & (ext[:, 2:] != 0),
                  ((0, 0), (2, 0)))  # [B,S] bool: s-2 -> s allowed
    fmax = G.max(2) - BOOST  # [B,T] per-frame fold
    _HOST["fmax_sum"] = fmax.sum(1)  # [B] exact compensation
    Gh = G - fmax[:, :, None]
    # fwd yq mask (yq[s] = y[s]*skip_ok[s+2]); bwd mask in reversed coords
    skf = np.full((B, S), NEG, np.float32)
    skf[:, :S - 2] = np.where(skip[:, 2:], 0.0, NEG)
    skb = np.where(skip[:, ::-1], 0.0, NEG).astype(np.float32)
    term = np.full((B, S), NEG, np.float32)
    term[np.arange(B), 2 * lens] = 0.0
    term[np.arange(B), 2 * lens - 1] = 0.0
    im = np.full((S,), NEG, np.float32)
    im[:2] = 0.0
    y0f = Gh[:, 0, :] + im[None, :]  # [B,S] alpha_0 logits
    y0b = (Gh[:, T - 1, :] + term)[:, ::-1]  # gamma_{T-1}, reversed s

    Wm = np.zeros((SB2, S), np.float32)  # z[f] = y[f] + y[f-1] + yq[f-2]
    for f in range(S):
        Wm[f, f] = 1.0
        if f >= 1:
            Wm[f - 1, f] = 1.0
        if f >= 2:
            Wm[S + f - 2, f] = 1.0
    Wm = np.concatenate([Wm, Wm], axis=1)  # duplicate z onto both halves

    pred8 = pred.astype(np_f8)
    in_maps = []
    for c in range(N_CORES):
        sl = slice(c * BS, (c + 1) * BS)
        Ghf = Gh[sl, 1:HS, :]  # [16,79,S] fwd frames t=1..79
        Ghb = Gh[sl, T - 2:HS - 1:-1, ::-1]  # [16,79,S] t=158..80, rev s
        skfc, skbc = skf[sl], skb[sl]
        x0 = np.empty((SB2, NCOL), np.float32)
        x0[0:S, 0:BS] = y0f[sl].T
        x0[0:S, BS:] = y0b[sl].T
        x0[S:, 0:BS] = (y0f[sl] + skfc).T
        x0[S:, BS:] = (y0b[sl] + skbc).T
        pq = np.empty((SB2, NSUP, NCOL), np.float32)
        pq[0:S, :, 0:BS] = Ghf.transpose(2, 1, 0)
        pq[0:S, :, BS:] = Ghb.transpose(2, 1, 0)
        pq[S:, :, 0:BS] = (Ghf + skfc[:, None, :]).transpose(2, 1, 0)
        pq[S:, :, BS:] = (Ghb + skbc[:, None, :]).transpose(2, 1, 0)
        in_maps.append({
            "pred": np.ascontiguousarray(pred8[sl]),
            "w": Wm,
            "x0": x0.astype(np.float16),
            "pq": np.ascontiguousarray(
                pq.reshape(SB2, NSUP * NCOL)).astype(np.float16),
        })
    return in_maps


def finish_host(results, lens):
    """Combine per-core outputs into the scalar mean loss (float64)."""
    fmax_sum = _HOST["fmax_sum"]
    loss_b = np.zeros(B, dtype=np.float64)
    with np.errstate(divide="ignore", invalid="ignore"):
        for c in range(N_CORES):
            r = results[c]
            acc = r["acc"].astype(np.float64)  # [128, NACC]
            ssum = np.empty((NBLK, 128))  # per-block row sums; row = b*8+t_off
            ssum[0] = acc[:, :NQCH].sum(-1)
            ssum[1:NBLK - 1] = acc[:, NQCH:NQCH + NBLK - 2].T
            ssum[NBLK - 1] = acc[:, NQCH + NBLK - 2:].sum(-1)
            lse = np.log(ssum)  # [NBLK, 128]
            s_lse = lse.reshape(NBLK, BS, TBLK).sum((0, 2))  # [BS]
            xf = r["xfin"].astype(np.float64)  # [SB2, NCOL]
            a79 = xf[0:S, 0:BS]  # [S,16] alpha_79
            g80 = xf[0:S, BS:][::-1, :]  # gamma_80[s]
            gq80 = xf[S:, BS:][::-1, :]  # gamma_80[s]*skip_ok[s]
            beta = g80.copy()
            beta[:-1] += g80[1:]
            beta[:-2] += gq80[2:]
            P = (a79 * beta).sum(0)  # [16]
            sl = slice(c * BS, (c + 1) * BS)
            logP = np.log(P) + fmax_sum[sl]
            loss_b[sl] = s_lse - logP
    loss_b = np.where(loss_b >= 1e29, 0.0, loss_b)
    loss_b = np.where(np.isfinite(loss_b), loss_b, 0.0)
    loss = np.mean(loss_b / np.maximum(lens.astype(np.float64), 1.0))
    return np.float32(loss)


def kernel(pred, targets, targets_lengths):
    pred = np.asarray(pred, dtype=np.float32)
    targets = np.asarray(targets).astype(np.int64)
    lens = np.asarray(targets_lengths).astype(np.int64)

    nc = _build_program()
    in_maps = prepare_in_maps(pred, targets, lens)
    res = run_bass_kernel_spmd(nc, in_maps, core_ids=list(range(N_CORES)))
    return finish_host(res.results, lens)


# revision 23
# speedup vs baseline: 1.5368x; 1.1951x over previous
"""CTC loss kernel for Trainium2 (8 NeuronCores, data-parallel over batch).

Math: with raw logits G[b,t,s] = pred[b,t,ext[b,s]] (ext = blank-interleaved
targets) the CTC forward recursion commutes with the per-frame log-softmax
normalizer: running the recursion on raw logits and subtracting
sum_t logsumexp_c(pred[b,t,:]) at the end gives the same loss.

Device work, per core (16 samples):
1. sum_c exp(pred) per (b,t): fp8(e4m3) pred streamed via HWDGE, ACT
   exp+accumulate (ACT-throughput bound; fp8 host-cast keeps the DMA side
   far under the SBUF fabric share).
2. The CTC recursion in probability space, forward and backward chains in
   lockstep columns of one [102, 32] state: per super-step one PE matmul
   z = W^T X (W = [K1; K2] encodes the +0/+1/+2 state shifts, identical for
   both chains because the backward state is stored index-reversed) and two
   DVE multiplies by host-prepacked exp'd logits. Per-frame max-logit folding
   (host) replaces renormalization entirely; host compensates exactly.
Host finishes the tiny join + scalar math in float64.
"""

import sys

sys.path.insert(0, "/opt/trn_rl_repo")

import ml_dtypes
import numpy as np

import concourse.bacc as bacc
import concourse.tile as tile
from concourse import mybir
from concourse.bass_utils import run_bass_kernel_spmd

B, T, C, L = 128, 160, 6625, 25
S = 2 * L + 1  # 51 CTC states
SB2 = 2 * S  # 102: [y; yq] stacked state rows
N_CORES = 8
BS = B // N_CORES  # 16 samples per core
NCOL = 2 * BS  # 32: fwd cols 0-15, bwd cols 16-31
HS = T // 2  # 80 frames per chain
NSUP = HS - 1  # 79 super-steps
TBLK = 8  # t-values per 128-row streaming block (8*16 = 128 rows)
NBLK = T // TBLK  # 20
# finer parts for the first/last streaming block: earlier pipeline start,
# smaller exposed tail.
QCHUNKS = [(0, 1657), (1657, 3313), (3313, 4969), (4969, 6625)]
NQCH = len(QCHUNKS)
QCHMAX = max(c1 - c0 for c0, c1 in QCHUNKS)
# one accumulator column per ACT accumulate: 4 chunks for block 0 (cols
# 0-3), one per middle block j=1..18 (col 3+j), 4 for block 19 (cols 22-25)
NACC = 2 * NQCH + (NBLK - 2)
NEG = -1.0e4  # exp() underflows to exactly 0.0f
BOOST = 0.5  # per-frame fold = fmax - BOOST: keeps fp32 range centered

f32 = mybir.dt.float32
f16 = mybir.dt.float16
f8 = mybir.dt.float8e4
Exp = mybir.ActivationFunctionType.Exp
np_f8 = ml_dtypes.float8_e4m3

_CACHE = {}
_HOST = {}


def _build_program():
    if "nc" in _CACHE:
        return _CACHE["nc"]
    nc = bacc.Bacc("TRN2", target_bir_lowering=False, debug=False,
                   num_devices=N_CORES)
    pred_d = nc.dram_tensor("pred", [BS, T, C], f8, kind="ExternalInput").ap()
    pred16_d = nc.dram_tensor("pred16", [BS, T, C], f16,
                              kind="ExternalInput").ap()
    w_d = nc.dram_tensor("w", [SB2, SB2], f32, kind="ExternalInput").ap()
    x0_d = nc.dram_tensor("x0", [SB2, NCOL], f16, kind="ExternalInput").ap()
    pq_d = nc.dram_tensor("pq", [SB2, NSUP * NCOL], f16,
                          kind="ExternalInput").ap()
    acc_d = nc.dram_tensor("acc", [128, NACC], f32,
                           kind="ExternalOutput").ap()
    xf_d = nc.dram_tensor("xfin", [SB2, NCOL], f32,
                          kind="ExternalOutput").ap()

    # pq column chunks (whole super-steps per chunk) for early recursion start
    PQCH = []
    st = 0
    for n in (20, 20, 20, 19):
        PQCH.append((st * NCOL, (st + n) * NCOL))
        st += n

    with tile.TileContext(nc) as tc:
        with (
            tc.tile_pool(name="persist", bufs=1) as pp,
            tc.tile_pool(name="stream", bufs=6) as spool,
            tc.tile_pool(name="zp", bufs=4, space="PSUM") as psp,
        ):
            # ---- recursion inputs, emitted first so PE/DVE start early
            wt = pp.tile([SB2, SB2], f32, tag="wt")
            x0l = pp.tile([SB2, NCOL], f16, tag="x0l")
            pqt = pp.tile([SB2, NSUP * NCOL], f16, tag="pqt")
            pqe = pp.tile([SB2, NSUP * NCOL], f32, tag="pqe")
            Xa = pp.tile([SB2, NCOL], f32, tag="Xa")
            Xb = pp.tile([SB2, NCOL], f32, tag="Xb")
            nc.sync.dma_start(out=wt[:], in_=w_d[:])
            nc.sync.dma_start(out=x0l[:], in_=x0_d[:])
            for a, b in PQCH:
                nc.sync.dma_start(out=pqt[:, a:b], in_=pq_d[:, a:b])
            nc.scalar.activation(Xa[:], x0l[:], Exp)
            for a, b in PQCH:
                nc.scalar.activation(pqe[:, a:b], pqt[:, a:b], Exp)

            # ---- lockstep fwd/bwd recursion: 79 x (1 matmul + 1 DVE mul).
            # The stationary matrix is [W | W] so the matmul lands z
            # duplicated on partitions 0-50 and 51-101 — DVE lanes cannot
            # cross partitions, so the y- and yq-halves each need z in their
            # own partitions; the duplicate makes the whole state update a
            # single partition-aligned multiply.
            cur, nxt = Xa, Xb
            for i in range(NSUP):
                z = psp.tile([SB2, NCOL], f32, tag="z")
                nc.tensor.matmul(z[:], wt[:], cur[:])
                c0 = i * NCOL
                nc.vector.tensor_mul(out=nxt[:], in0=z[:],
                                     in1=pqe[:, c0:c0 + NCOL])
                cur, nxt = nxt, cur

            # ---- streaming sum(exp(pred)) over C, 128 (b,t) rows per block.
            # pred is pre-cast to fp8 e4m3 on the host; all loads ride the
            # sync HWDGE ring (SWDGE would stall: its Q7 descriptor writes
            # arbitrate for the DVE/GpSimd shared SBUF port pair that the
            # recursion DVE holds most of the time; HWDGE is immune). The
            # stream is ACT-throughput bound (1 elem/cycle/lane), so the DMA
            # side has ample slack. exp output goes to one reused fp16
            # scratch (ACT engine port, free); accumulation stays fp32.
            # Every accumulate targets its own column of one persistent tile
            # and the whole accumulator ships in a single DMA at the end.
            # Mixed dtype per block: fp8 halves the DMA bytes but costs 1.2x
            # on ACT (6.98 vs 5.82 us/block measured); fp16 is ACT-cheap but
            # DMA-heavy (8.07 us/block at the ~217 GB/s fabric share).
            # Interleaving 8 fp8 middles among 10 fp16 ones keeps the DMA
            # stream ahead of ACT while ACT runs at the cheap mixed average.
            F8MID = {2, 4, 6, 8, 10, 12, 14, 16}
            accA = pp.tile([128, NACC], f32, tag="accA")
            scr = pp.tile([128, C], f16, tag="scr")
            for j in range(NBLK):
                if j in (0, NBLK - 1):
                    src = pred16_d[:, j * TBLK:(j + 1) * TBLK, :]
                    cb = 0 if j == 0 else NQCH + (NBLK - 2)
                    for ci, (c0, c1) in enumerate(QCHUNKS):
                        w = c1 - c0
                        cp = spool.tile([128, QCHMAX], f16, tag="chunkpart")
                        nc.sync.dma_start(out=cp[:, :w],
                                          in_=src[:, :, c0:c1])
                        nc.scalar.activation(scr[:, :w], cp[:, :w], Exp,
                                             accum_out=accA[:, cb + ci:
                                                            cb + ci + 1])
                elif j in F8MID:
                    src = pred_d[:, j * TBLK:(j + 1) * TBLK, :]
                    ct = spool.tile([128, C], f8, tag="chunk8")
                    nc.sync.dma_start(out=ct[:], in_=src)
                    nc.scalar.activation(scr[:], ct[:], Exp,
                                         accum_out=accA[:, NQCH + j - 1:
                                                        NQCH + j])
                else:
                    src = pred16_d[:, j * TBLK:(j + 1) * TBLK, :]
                    ct = spool.tile([128, C], f16, tag="chunk16")
                    nc.sync.dma_start(out=ct[:], in_=src)
                    nc.scalar.activation(scr[:], ct[:], Exp,
                                         accum_out=accA[:, NQCH + j - 1:
                                                        NQCH + j])
            nc.sync.dma_start(out=acc_d[:], in_=accA[:])
            # recursion result ships via the otherwise-idle SWDGE queue: on
            # the sync ring the scheduler hoists it ahead of later stream
            # DMAs and its recursion-end wait head-of-line-blocks them
            nc.gpsimd.dma_start(out=xf_d[:], in_=cur[:])

    nc.compile()
    _CACHE["nc"] = nc
    return nc


def prepare_in_maps(pred, targets, lens):
    """Host prep: gathered+folded logit packs, per-core sharding."""
    ext = np.zeros((B, S), dtype=np.int64)
    ext[:, 1::2] = targets
    G = pred[np.arange(B)[:, None, None], np.arange(T)[None, :, None],
             ext[:, None, :]]  # [B, T, S]
    valid = np.arange(S)[None, :] < (2 * lens + 1)[:, None]  # [B, S]
    G = np.where(valid[:, None, :], G, NEG).astype(np.float32)
    skip = np.pad((ext[:, 2:] != ext[:, :-2]) & (ext[:, 2:] != 0),
                  ((0, 0), (2, 0)))  # [B,S] bool: s-2 -> s allowed
    fmax = G.max(2) - BOOST  # [B,T] per-frame fold
    _HOST["fmax_sum"] = fmax.sum(1)  # [B] exact compensation
    Gh = G - fmax[:, :, None]
    # fwd yq mask (yq[s] = y[s]*skip_ok[s+2]); bwd mask in reversed coords
    skf = np.full((B, S), NEG, np.float32)
    skf[:, :S - 2] = np.where(skip[:, 2:], 0.0, NEG)
    skb = np.where(skip[:, ::-1], 0.0, NEG).astype(np.float32)
    term = np.full((B, S), NEG, np.float32)
    term[np.arange(B), 2 * lens] = 0.0
    term[np.arange(B), 2 * lens - 1] = 0.0
    im = np.full((S,), NEG, np.float32)
    im[:2] = 0.0
    y0f = Gh[:, 0, :] + im[None, :]  # [B,S] alpha_0 logits
    y0b = (Gh[:, T - 1, :] + term)[:, ::-1]  # gamma_{T-1}, reversed s

    Wm = np.zeros((SB2, S), np.float32)  # z[f] = y[f] + y[f-1] + yq[f-2]
    for f in range(S):
        Wm[f, f] = 1.0
        if f >= 1:
            Wm[f - 1, f] = 1.0
        if f >= 2:
            Wm[S + f - 2, f] = 1.0
    Wm = np.concatenate([Wm, Wm], axis=1)  # duplicate z onto both halves

    pred8 = pred.astype(np_f8)
    in_maps = []
    for c in range(N_CORES):
        sl = slice(c * BS, (c + 1) * BS)
        Ghf = Gh[sl, 1:HS, :]  # [16,79,S] fwd frames t=1..79
        Ghb = Gh[sl, T - 2:HS - 1:-1, ::-1]  # [16,79,S] t=158..80, rev s
        skfc, skbc = skf[sl], skb[sl]
        x0 = np.empty((SB2, NCOL), np.float32)
        x0[0:S, 0:BS] = y0f[sl].T
        x0[0:S, BS:] = y0b[sl].T
        x0[S:, 0:BS] = (y0f[sl] + skfc).T
        x0[S:, BS:] = (y0b[sl] + skbc).T
        pq = np.empty((SB2, NSUP, NCOL), np.float32)
        pq[0:S, :, 0:BS] = Ghf.transpose(2, 1, 0)
        pq[0:S, :, BS:] = Ghb.transpose(2, 1, 0)
        pq[S:, :, 0:BS] = (Ghf + skfc[:, None, :]).transpose(2, 1, 0)
        pq[S:, :, BS:] = (Ghb + skbc[:, None, :]).transpose(2, 1, 0)
        in_maps.append({
            "pred": np.ascontiguousarray(pred8[sl]),
            "pred16": np.ascontiguousarray(pred[sl]).astype(np.float16),
            "w": Wm,
            "x0": x0.astype(np.float16),
            "pq": np.ascontiguousarray(
                pq.reshape(SB2, NSUP * NCOL)).astype(np.float16),
        })
    return in_maps


def finish_host(results, lens):
    """Combine per-core outputs into the scalar mean loss (float64)."""
    fmax_sum = _HOST["fmax_sum"]
    loss_b = np.zeros(B, dtype=np.float64)
    with np.errstate(divide="ignore", invalid="ignore"):
        for c in range(N_CORES):
            r = results[c]
            acc = r["acc"].astype(np.float64)  # [128, NACC]
            ssum = np.empty((NBLK, 128))  # per-block row sums; row = b*8+t_off
            ssum[0] = acc[:, :NQCH].sum(-1)
            ssum[1:NBLK - 1] = acc[:, NQCH:NQCH + NBLK - 2].T
            ssum[NBLK - 1] = acc[:, NQCH + NBLK - 2:].sum(-1)
            lse = np.log(ssum)  # [NBLK, 128]
            s_lse = lse.reshape(NBLK, BS, TBLK).sum((0, 2))  # [BS]
            xf = r["xfin"].astype(np.float64)  # [SB2, NCOL]
            a79 = xf[0:S, 0:BS]  # [S,16] alpha_79
            g80 = xf[0:S, BS:][::-1, :]  # gamma_80[s]
            gq80 = xf[S:, BS:][::-1, :]  # gamma_80[s]*skip_ok[s]
            beta = g80.copy()
            beta[:-1] += g80[1:]
            beta[:-2] += gq80[2:]
            P = (a79 * beta).sum(0)  # [16]
            sl = slice(c * BS, (c + 1) * BS)
            logP = np.log(P) + fmax_sum[sl]
            loss_b[sl] = s_lse - logP
    loss_b = np.where(loss_b >= 1e29, 0.0, loss_b)
    loss_b = np.where(np.isfinite(loss_b), loss_b, 0.0)
    loss = np.mean(loss_b / np.maximum(lens.astype(np.float64), 1.0))
    return np.float32(loss)


def kernel(pred, targets, targets_lengths):
    pred = np.asarray(pred, dtype=np.float32)
    targets = np.asarray(targets).astype(np.int64)
    lens = np.asarray(targets_lengths).astype(np.int64)

    nc = _build_program()
    in_maps = prepare_in_maps(pred, targets, lens)
    res = run_bass_kernel_spmd(nc, in_maps, core_ids=list(range(N_CORES)))
    return finish_host(res.results, lens)
